# revision 1
# baseline (speedup 1.0000x reference)
"""Trainium2 Bass kernel for nn_CellAnnotator (per-pixel 8x8 locally-connected
weighted pooling with normalization), SPMD across 8 NeuronCores.

Contract: kernel(**inputs) takes FULL inputs (x0 [512,512,128] f32,
weights [512,512,64] f32, cnts [512,512,1] f32) and returns the FULL
output [512,512,128] f32.

Sharding: rows (H) split across 8 cores, 64 output rows each; each core's
input shard carries a 3+4-row halo (built host-side, zero-padded at the
image borders), so no device-to-device communication is needed.

Algorithm (banded matmul on the TensorEngine):
  out[i,j,c] = sum_{p,q} w[i,j,p*8+q] * x_pad[i+p, j+q, c], normalized by
  the same pooling applied to cnts (appended as channel 128 of x).
  For an output row i and a 57-pixel column block, the 64-tap sum is done as
  4 PSUM-accumulated matmuls, one per input-row pair: contraction dim
  K = 128 = (2 rows) x (64 input cols); lhsT is a banded [128, 57] weight
  tile (built host-side: w[i,j,p*8+q] sits at [64*(p%2) + (j-j0) + q, j-j0]);
  rhs is the bf16 input tile [128, 129] (128 channels + cnts).

DMA batching: input tiles for one row-pair are loaded for all 9 column
blocks in 2 DMAs; band tiles come as one 525KB DMA per row; the output row
is staged in one SBUF tile (blocked pixel layout, de-blocked on host) and
stored with a single DMA.
"""

import numpy as np
import ml_dtypes
from contextlib import ExitStack

import concourse.bass as bass
import concourse.bacc as bacc
import concourse.mybir as mybir
import concourse.tile as tile
from concourse.ap import AP
from concourse.bass_utils import run_bass_kernel_spmd

BF16 = np.dtype(ml_dtypes.bfloat16)

# Problem constants (hardcoded per contract)
H, W, C = 512, 512, 128
ROI = 8
TAPS = ROI * ROI
PAD_LO, PAD_HI = 3, 4          # XLA SAME padding for even kernel
NCORES = 8
ROWS = H // NCORES             # 64 output rows per core
IN_ROWS = ROWS + ROI - 1       # 71 input rows (halo included)
WPAD = W + ROI                 # padded width: cols -3 .. 516 (520)
CCH = C + 1                    # x channels + cnts as channel 128

BLK = 57                       # output pixels per column block
NBLK = 9                       # 8*57 + 56 = 512
PPAIRS = 4                     # input-row pairs per output row
BFREE = PPAIRS * NBLK * BLK    # free size of one row's band line (2052)

_CACHE = {}


def _build_nc(rep=1, variant="full"):
    f32 = mybir.dt.float32
    bf = mybir.dt.bfloat16
    nc = bacc.Bacc("TRN2", target_bir_lowering=False, debug=False,
                   num_devices=NCORES)
    # pre-stacked pair tiles: xcp[ri, 64g+u, b, c] = xpad[ri+g, 57b + u, c]
    NPAIRS = IN_ROWS - 1
    xcp = nc.dram_tensor("xcp", [NPAIRS, 128, NBLK, CCH], bf,
                         kind="ExternalInput")
    bnd = nc.dram_tensor("bnd", [ROWS, 128, BFREE], bf, kind="ExternalInput")
    # blocked output layout: [row, jj, b, c]; de-blocked + upcast host-side
    out = nc.dram_tensor("out", [ROWS, BLK, NBLK, C], bf,
                         kind="ExternalOutput")

    with tile.TileContext(nc) as tc:
        with ExitStack() as ctx:
            if rep > 1:
                ctx.enter_context(tc.For_i(0, rep, 1))
            xpool = ctx.enter_context(tc.tile_pool(name="xp", bufs=12))
            bpool = ctx.enter_context(tc.tile_pool(name="bp", bufs=3))
            ppool = ctx.enter_context(
                tc.tile_pool(name="pp", bufs=8, space="PSUM"))
            opool = ctx.enter_context(tc.tile_pool(name="op", bufs=3))
            spool = ctx.enter_context(tc.tile_pool(name="sp", bufs=4))

            xcache = {}

            def get_x(ri):
                """Input tiles for row pair (ri, ri+1), all column blocks:
                [128 = 2x64 positions, 9 blocks, 129 channels]."""
                if ri not in xcache:
                    t = xpool.tile([128, NBLK, CCH], bf, tag="xt")
                    fr = NBLK * CCH
                    src = AP(xcp, ri * 128 * fr, [[fr, 128], [1, fr]])
                    nc.sync.dma_start(t[:], src)
                    xcache[ri] = t
                return xcache[ri]

            if variant in ("dmax", "dmao", "dmao4"):
                # X-only / out-only DMA probes
                if variant == "dmao4":
                    ot4 = opool.tile([BLK, 4, NBLK * C], bf, tag="ot4")
                    nc.vector.memset(ot4[:], 0.0)
                    fo = NBLK * C
                    for il in range(0, ROWS, 4):
                        dst = AP(out, il * BLK * fo,
                                 [[fo, BLK], [BLK * fo, 4], [1, fo]])
                        nc.sync.dma_start(dst, ot4[:])
                else:
                    ot0 = opool.tile([BLK, NBLK, C], bf, tag="ot")
                    nc.vector.memset(ot0[:], 0.0)
                    for il in range(ROWS):
                        if variant == "dmax":
                            for pp in range(PPAIRS):
                                get_x(il + 2 * pp)
                        else:
                            nc.sync.dma_start(out[il], ot0[:])
                _finish = True
            else:
                _finish = False
            OBATCH = 4
            fo = NBLK * C
            otile = None
            btile0 = None
            if variant == "nodma":
                btile0 = bpool.tile([128, PPAIRS, NBLK, BLK], bf, tag="bt")
                nc.sync.dma_start(
                    btile0[:], AP(bnd, 0, [[BFREE, 128], [1, BFREE]]))
            for il in range(ROWS if not _finish else 0):
                if variant == "nodma":
                    btile = btile0
                else:
                    btile = bpool.tile([128, PPAIRS, NBLK, BLK], bf,
                                       tag="bt")
                    nc.sync.dma_start(
                        btile[:],
                        AP(bnd, il * 128 * BFREE, [[BFREE, 128], [1, BFREE]]))
                if variant == "dmab":
                    continue
                if il % OBATCH == 0:
                    otile = opool.tile([BLK, OBATCH, fo], bf, tag="ot")
                if variant == "dma":
                    for pp in range(PPAIRS):
                        get_x(il + 2 * pp)
                    continue
                for b in range(NBLK):
                    m = W - BLK * b if b == NBLK - 1 else BLK
                    psum = ppool.tile([BLK, CCH], f32, tag="ps")
                    for pp in range(PPAIRS):
                        xt = get_x((il + 2 * pp) % 8 if variant == "nodma"
                                   else il + 2 * pp)
                        nc.tensor.matmul(
                            psum[:m, :], btile[:, pp, b, :m], xt[:, b, :],
                            start=(pp == 0), stop=(pp == PPAIRS - 1))
                    if variant == "pe":
                        nc.vector.tensor_copy(otile[:m, b, :], psum[:m, 0:C])
                        continue
                    rec = spool.tile([BLK, 1], f32, tag="rec")
                    nc.vector.tensor_scalar_add(
                        rec[:m, :], psum[:m, C:C + 1], 1e-6)
                    nc.vector.reciprocal(rec[:m, :], rec[:m, :])
                    odst = otile[:m, il % OBATCH, b * C:(b + 1) * C]
                    if b % 2 == 0:
                        nc.vector.tensor_scalar(
                            odst, psum[:m, 0:C], rec[:m, 0:1], None,
                            op0=mybir.AluOpType.mult)
                    else:
                        nc.scalar.activation(
                            odst, psum[:m, 0:C],
                            mybir.ActivationFunctionType.Identity,
                            scale=rec[:m, 0:1])
                if il % OBATCH == OBATCH - 1:
                    dst = AP(out, (il - OBATCH + 1) * BLK * fo,
                             [[fo, BLK], [BLK * fo, OBATCH], [1, fo]])
                    nc.scalar.dma_start(dst, otile[:])
    nc.compile()
    return nc


def _get_nc(rep=1, variant="full"):
    key = ("nc", rep, variant)
    if key not in _CACHE:
        _CACHE[key] = _build_nc(rep, variant)
    return _CACHE[key]


def _build_bands(weights):
    """bands[i, kpos, pp, b, jj] = w[i, 57b+jj, (2pp+g)*8 + d] at
    kpos = 64g + jj + d  (g = kpos//64, d in [0,8)); zero elsewhere."""
    wq = weights.reshape(H, W, ROI, ROI)          # [i, j, p, q]
    bands = np.zeros((H, 128, PPAIRS, NBLK, BLK), BF16)
    for b in range(NBLK):
        m = W - BLK * b if b == NBLK - 1 else BLK
        jv = np.arange(m)
        wb = wq[:, BLK * b:BLK * b + m]            # [H, m, 8, 8]
        for pp in range(PPAIRS):
            for g in range(2):
                p = 2 * pp + g
                for d in range(ROI):
                    bands[:, 64 * g + jv + d, pp, b, jv] = \
                        wb[:, jv, p, d].astype(BF16)
    return bands


def _host_prep(x0, weights, cnts):
    xcp = np.zeros((H + ROI - 1, WPAD, CCH), BF16)
    xcp[PAD_LO:PAD_LO + H, PAD_LO:PAD_LO + W, :C] = x0.astype(BF16)
    xcp[PAD_LO:PAD_LO + H, PAD_LO:PAD_LO + W, C] = cnts[:, :, 0].astype(BF16)
    # pre-stacked pair tiles: xb[ri, 64g+u, b, c] = xcp[ri+g, 57b + u, c]
    # (overlapping 64-wide windows at stride 57; rows duplicated so each
    # pair tile is one fully-contiguous DMA)
    sr, sc, sch = xcp.strides
    xb = np.lib.stride_tricks.as_strided(
        xcp, shape=(H + ROI - 2, 2, 64, NBLK, CCH),
        strides=(sr, sr, sc, BLK * sc, sch)).reshape(
            H + ROI - 2, 128, NBLK, CCH)
    bands = _build_bands(weights)
    in_maps = []
    for k in range(NCORES):
        r0 = k * ROWS
        in_maps.append({
            "xcp": np.ascontiguousarray(xb[r0:r0 + IN_ROWS - 1]),
            "bnd": np.ascontiguousarray(
                bands[r0:r0 + ROWS].reshape(ROWS, 128, BFREE)),
        })
    return in_maps


def _unblock(arr):
    """[ROWS, BLK, NBLK, C] blocked -> [ROWS, W, C] f32."""
    return arr.astype(np.float32).transpose(0, 2, 1, 3).reshape(
        ROWS, NBLK * BLK, C)[:, :W]


def kernel(x0, weights, cnts):
    x0 = np.asarray(x0, np.float32)
    weights = np.asarray(weights, np.float32)
    cnts = np.asarray(cnts, np.float32)
    nc = _get_nc()
    in_maps = _host_prep(x0, weights, cnts)
    res = run_bass_kernel_spmd(nc, in_maps, core_ids=list(range(NCORES)))
    return np.ascontiguousarray(np.concatenate(
        [_unblock(res.results[k]["out"]) for k in range(NCORES)], axis=0))



# revision 2
# speedup vs baseline: 2.5312x; 2.5312x over previous
"""Trainium2 Bass kernel for nn_CellAnnotator (per-pixel 8x8 locally-connected
weighted pooling with normalization), SPMD across 8 NeuronCores.

Contract: kernel(**inputs) takes FULL inputs (x0 [512,512,128] f32,
weights [512,512,64] f32, cnts [512,512,1] f32) and returns the FULL
output [512,512,128] f32.

Sharding: rows (H) split across 8 cores, 64 output rows each; each core's
input shard carries a 3+4-row halo (built host-side, zero-padded at the
image borders), so no device-to-device communication is needed.

Normalization is folded into the weights on the host: since the pooling is
linear in the per-pixel weights,
  att(x0, w) / (att(cnts, w) + eps) == att(x0, w / (att(cnts, w) + eps)),
so the host computes t = att(cnts, w) with 64 shifted-window numpy MACs,
scales the weights, and the device runs a single unnormalized pooling —
no cnts channel and no per-pixel divide on device.

Algorithm (banded matmul on the TensorEngine): for output row i and a
57-pixel column block, the 64-tap sum is 4 PSUM-accumulated matmuls, one
per input-row pair: contraction K = 128 = (2 rows) x (64 input cols).
The x pair tile [128, 128 ch] is the STATIONARY operand (128 columns ->
fast weight load), the banded weight tile [128, 57] is the moving operand
(built host-side: ws[i,j,p*8+q] at [64*(p%2) + (j-j0) + q, j-j0]), so the
PSUM result is [128 ch, 57 px], copied to bf16 and stored channel-major;
the host transposes back.
"""

import numpy as np
import ml_dtypes
from contextlib import ExitStack

import concourse.bass as bass
import concourse.bacc as bacc
import concourse.mybir as mybir
import concourse.tile as tile
from concourse.ap import AP
from concourse.bass_utils import run_bass_kernel_spmd

BF16 = np.dtype(ml_dtypes.bfloat16)

# Problem constants (hardcoded per contract)
H, W, C = 512, 512, 128
ROI = 8
PAD_LO, PAD_HI = 3, 4          # XLA SAME padding for even kernel
NCORES = 8
ROWS = H // NCORES             # 64 output rows per core
IN_ROWS = ROWS + ROI - 1       # 71 input rows (halo included)
WPAD = W + ROI                 # padded width: cols -3 .. 516 (520)

BLK = 57                       # output pixels per column block
NBLK = 9                       # 8*57 + 56 = 512
NJ = NBLK * BLK                # 513 blocked output columns per row
PPAIRS = 4                     # input-row pairs per output row
BFREE = PPAIRS * NBLK * BLK    # free size of one row's band line (2052)
NPAIRS = IN_ROWS - 1           # 70 pair tiles per core
BGRP = 3                       # column blocks per PSUM tile / copy

_CACHE = {}


def _build_nc(rep=1, variant="full"):
    f32 = mybir.dt.float32
    bf = mybir.dt.bfloat16
    nc = bacc.Bacc("TRN2", target_bir_lowering=False, debug=False,
                   num_devices=NCORES)
    # pre-stacked pair tiles: xcp[ri, 64g+u, b, c] = xpad[ri+g, 57b + u, c]
    xcp = nc.dram_tensor("xcp", [NPAIRS, 128, NBLK, C], bf,
                         kind="ExternalInput")
    bnd = nc.dram_tensor("bnd", [ROWS, 128, BFREE], bf, kind="ExternalInput")
    # channel-major blocked output: [row, c, 57b+jj]; transposed host-side
    out = nc.dram_tensor("out", [ROWS, C, NJ], bf, kind="ExternalOutput")

    with tile.TileContext(nc) as tc:
        with ExitStack() as ctx:
            if rep > 1:
                ctx.enter_context(tc.For_i(0, rep, 1))
            xpool = ctx.enter_context(tc.tile_pool(name="xp", bufs=14))
            bpool = ctx.enter_context(tc.tile_pool(name="bp", bufs=8))
            ppool = ctx.enter_context(
                tc.tile_pool(name="pp", bufs=6, space="PSUM"))
            opool = ctx.enter_context(tc.tile_pool(name="op", bufs=3))

            xcache = {}

            def get_x(ri):
                """Input tile for row pair (ri, ri+1), all column blocks:
                [128 = 2x64 positions, 9 blocks, 128 channels]."""
                if ri not in xcache:
                    t = xpool.tile([128, NBLK, C], bf, tag="xt")
                    fr = NBLK * C
                    src = AP(xcp, ri * 128 * fr, [[fr, 128], [1, fr]])
                    nc.scalar.dma_start(t[:], src)
                    xcache[ri] = t
                return xcache[ri]

            if variant in ("dmax", "dmao"):
                if variant == "dmao":
                    ot0 = opool.tile([128, 4, NJ], bf, tag="ot")
                    nc.vector.memset(ot0[:], 0.0)
                    fo = C * NJ
                    for il in range(0, ROWS, 4):
                        dst = AP(out, il * fo,
                                 [[NJ, 128], [fo, 4], [1, NJ]])
                        nc.gpsimd.dma_start(dst, ot0[:])
                else:
                    for il in range(ROWS):
                        for pp in range(PPAIRS):
                            get_x(il + 2 * pp)
                _finish = True
            else:
                _finish = False
            OBATCH = 4
            fo = C * NJ
            otile = None
            btile0 = None
            if variant == "nodma":
                btile0 = bpool.tile([128, PPAIRS, NBLK, BLK], bf, tag="bt")
                nc.sync.dma_start(
                    btile0[:], AP(bnd, 0, [[BFREE, 128], [1, BFREE]]))
            for il in range(ROWS if not _finish else 0):
                if variant == "nodma":
                    btile = btile0
                else:
                    btile = bpool.tile([128, PPAIRS, NBLK, BLK], bf,
                                       tag="bt")
                    nc.sync.dma_start(
                        btile[:],
                        AP(bnd, il * 128 * BFREE, [[BFREE, 128], [1, BFREE]]))
                if variant == "dmab":
                    continue
                if il % OBATCH == 0:
                    otile = opool.tile([128, OBATCH, NJ], bf, tag="ot")
                if variant == "dma":
                    for pp in range(PPAIRS):
                        get_x(il + 2 * pp)
                    continue
                for bg in range(NBLK // BGRP):
                    psum = ppool.tile([128, BGRP * BLK], f32, tag="ps")
                    for bb in range(BGRP):
                        b = bg * BGRP + bb
                        for pp in range(PPAIRS):
                            xt = get_x((il + 2 * pp) % 8 if variant == "nodma"
                                       else il + 2 * pp)
                            nc.tensor.matmul(
                                psum[:, bb * BLK:(bb + 1) * BLK],
                                xt[:, b, :], btile[:, pp, b, :],
                                start=(pp == 0), stop=(pp == PPAIRS - 1))
                    if variant == "pe":
                        continue
                    nc.vector.tensor_copy(
                        otile[:, il % OBATCH,
                              bg * BGRP * BLK:(bg + 1) * BGRP * BLK],
                        psum[:])
                if variant == "pe":
                    continue
                if il % OBATCH == OBATCH - 1:
                    dst = AP(out, (il - OBATCH + 1) * fo,
                             [[NJ, 128], [fo, OBATCH], [1, NJ]])
                    nc.gpsimd.dma_start(dst, otile[:])
    nc.compile()
    return nc


def _get_nc(rep=1, variant="full"):
    key = ("nc", rep, variant)
    if key not in _CACHE:
        _CACHE[key] = _build_nc(rep, variant)
    return _CACHE[key]


def _build_bands(weights):
    """bands[i, kpos, pp, b, jj] = w[i, 57b+jj, (2pp+g)*8 + d] at
    kpos = 64g + jj + d  (g = kpos//64, d in [0,8)); zero elsewhere."""
    wq = weights.reshape(H, W, ROI, ROI)          # [i, j, p, q]
    bands = np.zeros((H, 128, PPAIRS, NBLK, BLK), BF16)
    for b in range(NBLK):
        m = W - BLK * b if b == NBLK - 1 else BLK
        jv = np.arange(m)
        wb = wq[:, BLK * b:BLK * b + m]            # [H, m, 8, 8]
        for pp in range(PPAIRS):
            for g in range(2):
                p = 2 * pp + g
                for d in range(ROI):
                    bands[:, 64 * g + jv + d, pp, b, jv] = \
                        wb[:, jv, p, d].astype(BF16)
    return bands


def _host_prep(x0, weights, cnts):
    # fold normalization: t = att(cnts, w); ws = w / (t + 1e-6)
    cp = np.zeros((H + ROI - 1, W + ROI - 1), np.float32)
    cp[PAD_LO:PAD_LO + H, PAD_LO:PAD_LO + W] = cnts[:, :, 0]
    wq = weights.reshape(H, W, ROI * ROI)
    t = np.zeros((H, W), np.float32)
    for p in range(ROI):
        for q in range(ROI):
            t += wq[:, :, p * ROI + q] * cp[p:p + H, q:q + W]
    ws = weights / (t + 1e-6)[:, :, None]

    xcp = np.zeros((H + ROI - 1, WPAD, C), BF16)
    xcp[PAD_LO:PAD_LO + H, PAD_LO:PAD_LO + W] = x0.astype(BF16)
    # pre-stacked pair tiles: xb[ri, 64g+u, b, c] = xcp[ri+g, 57b + u, c]
    # (overlapping 64-wide windows at stride 57; rows duplicated so each
    # pair tile is one fully-contiguous DMA)
    sr, sc, sch = xcp.strides
    xb = np.lib.stride_tricks.as_strided(
        xcp, shape=(H + ROI - 2, 2, 64, NBLK, C),
        strides=(sr, sr, sc, BLK * sc, sch)).reshape(
            H + ROI - 2, 128, NBLK, C)
    bands = _build_bands(ws)
    in_maps = []
    for k in range(NCORES):
        r0 = k * ROWS
        in_maps.append({
            "xcp": np.ascontiguousarray(xb[r0:r0 + NPAIRS]),
            "bnd": np.ascontiguousarray(
                bands[r0:r0 + ROWS].reshape(ROWS, 128, BFREE)),
        })
    return in_maps


def _unblock(arr):
    """[ROWS, C, NJ] channel-major blocked -> [ROWS, W, C] f32."""
    return arr.astype(np.float32)[:, :, :W].transpose(0, 2, 1)


def kernel(x0, weights, cnts):
    x0 = np.asarray(x0, np.float32)
    weights = np.asarray(weights, np.float32)
    cnts = np.asarray(cnts, np.float32)
    nc = _get_nc()
    in_maps = _host_prep(x0, weights, cnts)
    res = run_bass_kernel_spmd(nc, in_maps, core_ids=list(range(NCORES)))
    return np.ascontiguousarray(np.concatenate(
        [_unblock(res.results[k]["out"]) for k in range(NCORES)], axis=0))


# revision 10
# speedup vs baseline: 4.6171x; 1.8241x over previous
"""Trainium2 Bass kernel for nn_CellAnnotator (per-pixel 8x8 locally-connected
weighted pooling with normalization), SPMD across 8 NeuronCores.

Contract: kernel(**inputs) takes FULL inputs (x0 [512,512,128] f32,
weights [512,512,64] f32, cnts [512,512,1] f32) and returns the FULL
output [512,512,128] f32.

Sharding: rows (H) split across 8 cores, 64 output rows each; each core's
input shard carries a 3+4-row halo (built host-side, zero-padded at the
image borders), so no device-to-device communication is needed.

Normalization is folded into the weights on the host: since the pooling is
linear in the per-pixel weights,
  att(x0, w) / (att(cnts, w) + eps) == att(x0, w / (att(cnts, w) + eps)),
so the host computes t = att(cnts, w) with 64 shifted-window numpy MACs,
scales the weights, and the device runs a single unnormalized pooling —
no cnts channel and no per-pixel divide on device.

Algorithm (banded matmul on the TensorEngine): for output row i and a
57-pixel column block, the 64-tap sum is 4 PSUM-accumulated matmuls, one
per input-row pair: contraction K = 128 = (2 rows) x (64 input cols).
The x pair tile [128, 128 ch] is the STATIONARY operand (128 columns ->
fast weight load), the banded weight tile [128, 57] is the moving operand
(built host-side: ws[i,j,p*8+q] at [64*(p%2) + (j-j0) + q, j-j0]), so the
PSUM result is [128 ch, 57 px], copied to bf16 and stored channel-major;
the host transposes back.
"""

import numpy as np
import ml_dtypes
from contextlib import ExitStack

import concourse.bass as bass
import concourse.bacc as bacc
import concourse.mybir as mybir
import concourse.tile as tile
from concourse.ap import AP
from concourse.bass_utils import run_bass_kernel_spmd

BF16 = np.dtype(ml_dtypes.bfloat16)

# Problem constants (hardcoded per contract)
H, W, C = 512, 512, 128
ROI = 8
PAD_LO, PAD_HI = 3, 4          # XLA SAME padding for even kernel
NCORES = 8
ROWS = H // NCORES             # 64 output rows per core
IN_ROWS = ROWS + ROI - 1       # 71 input rows (halo included)
WPAD = W + ROI                 # padded width: cols -3 .. 516 (520)

BLK = 57                       # output pixels per column block
NBLK = 9                       # 8*57 + 56 = 512
NJ = NBLK * BLK                # 513 blocked output columns per row
NJP = 516                      # padded row pitch (64B-aligned 8-row lines)
PPAIRS = 4                     # input-row pairs per output row
BFREE = PPAIRS * NBLK * BLK    # free size of one row's band line (2052)
NPAIRS = IN_ROWS - 1           # 70 pair tiles per core
BGRP = 3                       # column blocks per PSUM tile / copy
OBATCH = 8                     # output rows per store DMA

_CACHE = {}


def _build_nc(rep=1, variant="full"):
    f32 = mybir.dt.float32
    bf = mybir.dt.bfloat16
    nc = bacc.Bacc("TRN2", target_bir_lowering=False, debug=False,
                   num_devices=NCORES)
    # pre-stacked pair tiles: xcp[ri, 64g+u, b, c] = xpad[ri+g, 57b + u, c]
    xcp = nc.dram_tensor("xcp", [NPAIRS, 128, NBLK, C], bf,
                         kind="ExternalInput")
    bnd = nc.dram_tensor("bnd", [ROWS, 128, BFREE], bf, kind="ExternalInput")
    # channel-major blocked output: [c, row, 57b+jj]; transposed host-side.
    # c outermost makes an 8-row batch one contiguous 8256B line/partition.
    out = nc.dram_tensor("out", [C, ROWS, NJP], bf, kind="ExternalOutput")

    with tile.TileContext(nc) as tc:
        with ExitStack() as ctx:
            if rep > 1:
                ctx.enter_context(tc.For_i(0, rep, 1))
            xpool = ctx.enter_context(tc.tile_pool(name="xp", bufs=14))
            bpool = ctx.enter_context(tc.tile_pool(name="bp", bufs=8))
            ppool = ctx.enter_context(
                tc.tile_pool(name="pp", bufs=8, space="PSUM"))
            opool = ctx.enter_context(tc.tile_pool(name="op", bufs=3))

            xcache = {}

            def get_x(ri):
                """Input tile for row pair (ri, ri+1), all column blocks:
                [128 = 2x64 positions, 9 blocks, 128 channels]."""
                if ri not in xcache:
                    t = xpool.tile([128, NBLK, C], bf, tag="xt")
                    fr = NBLK * C
                    src = AP(xcp, ri * 128 * fr, [[fr, 128], [1, fr]])
                    nc.sync.dma_start(t[:], src)
                    xcache[ri] = t
                return xcache[ri]

            if variant in ("dmax", "dmao"):
                if variant == "dmao":
                    ot0 = opool.tile([128, OBATCH * NJP], bf, tag="ot")
                    nc.vector.memset(ot0[:], 0.0)
                    for il in range(0, ROWS, OBATCH):
                        dst = AP(out, il * NJP,
                                 [[ROWS * NJP, 128], [1, OBATCH * NJP]])
                        nc.scalar.dma_start(dst, ot0[:])
                else:
                    for il in range(ROWS):
                        for pp in range(PPAIRS):
                            get_x(il + 2 * pp)
                _finish = True
            else:
                _finish = False
            otile = None
            btile0 = None
            if variant == "nodma":
                btile0 = bpool.tile([128, PPAIRS, NBLK, BLK], bf, tag="bt")
                nc.sync.dma_start(
                    btile0[:], AP(bnd, 0, [[BFREE, 128], [1, BFREE]]))
            for il in range(ROWS if not _finish else 0):
                if variant == "nodma":
                    btile = btile0
                else:
                    btile = bpool.tile([128, PPAIRS, NBLK, BLK], bf,
                                       tag="bt")
                    nc.sync.dma_start(
                        btile[:],
                        AP(bnd, il * 128 * BFREE, [[BFREE, 128], [1, BFREE]]))
                if variant == "dmab":
                    continue
                if il % OBATCH == 0:
                    otile = opool.tile([128, OBATCH * NJP], bf, tag="ot")
                if variant == "dma":
                    for pp in range(PPAIRS):
                        get_x(il + 2 * pp)
                    continue
                for bg in range(NBLK // BGRP):
                    psum = ppool.tile([128, BGRP * BLK], f32, tag="ps")
                    for bb in range(BGRP):
                        b = bg * BGRP + bb
                        for pp in range(PPAIRS):
                            xt = get_x((il + 2 * pp) % 8 if variant == "nodma"
                                       else il + 2 * pp)
                            nc.tensor.matmul(
                                psum[:, bb * BLK:(bb + 1) * BLK],
                                xt[:, b, :], btile[:, pp, b, :],
                                start=(pp == 0), stop=(pp == PPAIRS - 1))
                    if variant == "pe":
                        continue
                    odst = otile[:, (il % OBATCH) * NJP + bg * BGRP * BLK:
                                 (il % OBATCH) * NJP + (bg + 1) * BGRP * BLK]
                    if (3 * il + bg) % 2 == 0:
                        nc.vector.tensor_copy(odst, psum[:])
                    else:
                        nc.scalar.activation(
                            odst, psum[:],
                            mybir.ActivationFunctionType.Identity)
                if variant == "pe":
                    continue
                if il % OBATCH == OBATCH - 1:
                    dst = AP(out, (il - OBATCH + 1) * NJP,
                             [[ROWS * NJP, 128], [1, OBATCH * NJP]])
                    nc.scalar.dma_start(dst, otile[:])
    nc.compile()
    return nc


def _get_nc(rep=1, variant="full"):
    key = ("nc", rep, variant)
    if key not in _CACHE:
        _CACHE[key] = _build_nc(rep, variant)
    return _CACHE[key]


def _build_bands(weights):
    """bands[i, kpos, pp, b, jj] = w[i, 57b+jj, (2pp+g)*8 + d] at
    kpos = 64g + jj + d  (g = kpos//64, d in [0,8)); zero elsewhere."""
    wq = weights.reshape(H, W, ROI, ROI)          # [i, j, p, q]
    bands = np.zeros((H, 128, PPAIRS, NBLK, BLK), BF16)
    for b in range(NBLK):
        m = W - BLK * b if b == NBLK - 1 else BLK
        jv = np.arange(m)
        wb = wq[:, BLK * b:BLK * b + m]            # [H, m, 8, 8]
        for pp in range(PPAIRS):
            for g in range(2):
                p = 2 * pp + g
                for d in range(ROI):
                    bands[:, 64 * g + jv + d, pp, b, jv] = \
                        wb[:, jv, p, d].astype(BF16)
    return bands


def _host_prep(x0, weights, cnts):
    # fold normalization: t = att(cnts, w); ws = w / (t + 1e-6)
    cp = np.zeros((H + ROI - 1, W + ROI - 1), np.float32)
    cp[PAD_LO:PAD_LO + H, PAD_LO:PAD_LO + W] = cnts[:, :, 0]
    wq = weights.reshape(H, W, ROI * ROI)
    t = np.zeros((H, W), np.float32)
    for p in range(ROI):
        for q in range(ROI):
            t += wq[:, :, p * ROI + q] * cp[p:p + H, q:q + W]
    ws = weights / (t + 1e-6)[:, :, None]

    xcp = np.zeros((H + ROI - 1, WPAD, C), BF16)
    xcp[PAD_LO:PAD_LO + H, PAD_LO:PAD_LO + W] = x0.astype(BF16)
    # pre-stacked pair tiles: xb[ri, 64g+u, b, c] = xcp[ri+g, 57b + u, c]
    # (overlapping 64-wide windows at stride 57; rows duplicated so each
    # pair tile is one fully-contiguous DMA)
    sr, sc, sch = xcp.strides
    xb = np.lib.stride_tricks.as_strided(
        xcp, shape=(H + ROI - 2, 2, 64, NBLK, C),
        strides=(sr, sr, sc, BLK * sc, sch)).reshape(
            H + ROI - 2, 128, NBLK, C)
    bands = _build_bands(ws)
    in_maps = []
    for k in range(NCORES):
        r0 = k * ROWS
        in_maps.append({
            "xcp": np.ascontiguousarray(xb[r0:r0 + NPAIRS]),
            "bnd": np.ascontiguousarray(
                bands[r0:r0 + ROWS].reshape(ROWS, 128, BFREE)),
        })
    return in_maps


def _unblock(arr):
    """[C, ROWS, NJP] channel-major blocked -> [ROWS, W, C] f32."""
    return arr.astype(np.float32)[:, :, :W].transpose(1, 2, 0)


def kernel(x0, weights, cnts):
    x0 = np.asarray(x0, np.float32)
    weights = np.asarray(weights, np.float32)
    cnts = np.asarray(cnts, np.float32)
    nc = _get_nc()
    in_maps = _host_prep(x0, weights, cnts)
    res = run_bass_kernel_spmd(nc, in_maps, core_ids=list(range(NCORES)))
    return np.ascontiguousarray(np.concatenate(
        [_unblock(res.results[k]["out"]) for k in range(NCORES)], axis=0))
